# revision 16
# baseline (speedup 1.0000x reference)
"""Distributed Trainium2 kernel for nn_AtomicLinear.

Reference math:
    perm = softmax((logits + gumbel) / T, axis=-1)          # [128, 128]
    rowsum = perm.sum(-1)                                   # [128]
    out = einsum('bi,oi,i->bo', x, weight, rowsum) + bias   # [4096, 512]

softmax(z, axis=-1) rows sum to 1 by construction (the reference's own
rowsum is 1 +- 1e-7 float noise), so the contraction reduces exactly to
    out = x @ weight.T + bias.

Sharding: data-parallel over the batch axis of x -- each of the 8 cores
takes a 512-row shard of x, replicates weight/bias, and produces its
512-row shard of the output. No collectives.

Implementation notes:
  - x and weight are loaded with row group r = 4p + a on partition p /
    free block a: every per-partition DMA chunk is 2 KB contiguous.
    The row permutation is undone on the x side by a stride-4 row AP in
    the output DMA, and on the weight side by a free-dim-shuffled read
    AP in the DVE epilogue.
  - Inputs are cast f32 -> bf16 *during* the input DMA (SWDGE on
    GpSimd), which also keeps both HWDGE rings free.
  - Transposes run on the DMA xbar (2-byte transpose mode), not on the
    TensorEngine.
  - PE does one ones-outer-product matmul (bias broadcast, float32r,
    exact) + 4 bf16 matmuls. DVE does one copy + 4 epilogue adds.
  - matmul accumulation is fp32 in PSUM; bf16 operand rounding gives
    ~2e-3 relative error, well inside the 2e-2 gate.
"""

import numpy as np

import concourse.bass as bass
import concourse.mybir as mybir
from concourse.bacc import Bacc
from concourse.bass import ts
from concourse.bass_utils import run_bass_kernel_spmd
from concourse.tile import TileContext

N_CORES = 8
B, IN, OUT = 4096, 128, 512
B_SH = B // N_CORES  # 512 rows of x per core
P = 128
NT = B_SH // P  # 4 batch tiles per core
WT = OUT // P  # 4 weight tiles
F32 = mybir.dt.float32
F32R = mybir.dt.float32r
BF16 = mybir.dt.bfloat16

_CACHED_NC = None


def _build():
    nc = Bacc()

    x_ext = nc.declare_dram_parameter("x", [B_SH, IN], F32, isOutput=False)
    w_ext = nc.declare_dram_parameter("weight", [OUT, IN], F32, isOutput=False)
    b_ext = nc.declare_dram_parameter("bias", [OUT], F32, isOutput=False)
    out_ext = nc.declare_dram_parameter("out", [B_SH, OUT], F32, isOutput=True)

    # row r = 4p + a -> partition p, free block a (2KB contiguous / partition)
    x_blk = x_ext.rearrange("(p a) i -> p a i", a=NT)
    w_blk = w_ext.rearrange("(p a) i -> p a i", a=WT)
    out_blk = out_ext.rearrange("(p a) o -> p a o", a=NT)

    with TileContext(nc) as tc:
        with (
            tc.tile_pool(name="consts", bufs=1) as consts,
            tc.tile_pool(name="sbuf", bufs=1) as sbuf,
            tc.tile_pool(name="xtp", bufs=4) as xtp,
            tc.tile_pool(name="psum_b", bufs=1, space="PSUM") as psum_b_pool,
            tc.tile_pool(name="psum_out", bufs=3, space="PSUM") as psum_out,
            tc.tile_pool(name="outp", bufs=4) as outp,
        ):
            # ---- input DMAs: f32 -> bf16 cast on the SWDGE (GpSimd) path
            x_bf = sbuf.tile([P, NT, P], BF16)
            nc.gpsimd.dma_start(x_bf[:, 0:2, :], x_blk[:, 0:2, :])
            nc.gpsimd.dma_start(x_bf[:, 2:4, :], x_blk[:, 2:4, :])
            w_bf = sbuf.tile([P, WT, P], BF16)
            nc.gpsimd.dma_start(w_bf, w_blk)
            bias_sb = consts.tile([1, OUT], F32)
            nc.scalar.dma_start(bias_sb, b_ext[None, :])

            # ---- transposes on the DMA xbar (2-byte transpose mode) ----
            wT = sbuf.tile([P, WT * P], BF16)
            for t in range(WT):
                nc.scalar.dma_start(wT[:, ts(t, P)], w_bf[:, t, :], transpose=True)
            xTs = []
            for t in range(NT):
                xTt = xtp.tile([P, P], BF16)
                nc.sync.dma_start(xTt, x_bf[:, t, :], transpose=True)
                xTs.append(xTt)

            # ---- bias broadcast via ones-outer-product matmul (exact) ----
            of = consts.tile([1, P], F32)
            nc.gpsimd.memset(of, 1.0)
            ones_r = consts.tile([1, P], F32R)
            nc.vector.tensor_copy(ones_r, of)
            bias_r = consts.tile([1, OUT], F32R)
            nc.vector.tensor_copy(bias_r, bias_sb)
            psum_b = psum_b_pool.tile([P, OUT], F32)
            nc.tensor.matmul(psum_b, ones_r, bias_r, start=True, stop=True)
            bias_bc = consts.tile([P, OUT], F32)
            nc.vector.tensor_copy(bias_bc, psum_b)

            # ---- main matmuls + epilogue (read-AP undoes o = 4j + a) ----
            for t in range(NT):
                po = psum_out.tile([P, OUT], F32)
                nc.tensor.matmul(po, xTs[t], wT, start=True, stop=True)
                ot = outp.tile([P, OUT], F32)
                nc.vector.tensor_add(
                    ot.rearrange("p (j a) -> p j a", a=WT),
                    po.rearrange("p (a j) -> p j a", a=WT),
                    bias_bc.rearrange("p (j a) -> p j a", a=WT),
                )
                eng = nc.sync if t % 2 == 0 else nc.scalar
                eng.dma_start(out_blk[:, t, :], ot)

    nc.finalize()
    return nc


def get_nc():
    global _CACHED_NC
    if _CACHED_NC is None:
        _CACHED_NC = _build()
    return _CACHED_NC


def make_in_maps(x, weight, bias, logits, gumbel):
    x = np.ascontiguousarray(x, dtype=np.float32)
    weight = np.ascontiguousarray(weight, dtype=np.float32)
    bias = np.ascontiguousarray(bias, dtype=np.float32)
    return [
        {
            "x": np.ascontiguousarray(x[i * B_SH : (i + 1) * B_SH]),
            "weight": weight,
            "bias": bias,
        }
        for i in range(N_CORES)
    ]


def run(inputs, trace=False, **kwargs):
    nc = get_nc()
    in_maps = make_in_maps(**inputs)
    res = run_bass_kernel_spmd(
        nc, in_maps, core_ids=list(range(N_CORES)), trace=trace, **kwargs
    )
    out = np.concatenate(
        [np.asarray(res.results[i]["out"]) for i in range(N_CORES)], axis=0
    )
    return out.astype(np.float32), res


def kernel(**inputs):
    out, _ = run(inputs, trace=False)
    return out


# revision 17
# speedup vs baseline: 1.0791x; 1.0791x over previous
"""Distributed Trainium2 kernel for nn_AtomicLinear.

Reference math:
    perm = softmax((logits + gumbel) / T, axis=-1)          # [128, 128]
    rowsum = perm.sum(-1)                                   # [128]
    out = einsum('bi,oi,i->bo', x, weight, rowsum) + bias   # [4096, 512]

softmax(z, axis=-1) rows sum to 1 by construction (the reference's own
rowsum is 1 +- 1e-7 float noise), so the contraction reduces exactly to
    out = x @ weight.T + bias.

Sharding: data-parallel over the batch axis of x -- each of the 8 cores
takes a 512-row shard of x, replicates weight/bias, and produces its
512-row shard of the output. No collectives.

Implementation notes:
  - x and weight are loaded with row group r = 4p + a on partition p /
    free block a: every per-partition DMA chunk is 2 KB contiguous.
    The row permutation is undone on the x side by a stride-4 row AP in
    the output DMA, and on the weight side by a free-dim-shuffled read
    AP in the wT cast.
  - weight rides the sync HWDGE ring alone (it gates all matmuls);
    bias + the two x halves ride the scalar ring.
  - bias broadcast [1,512] -> [128,512] via a ones-outer-product
    float32r matmul (exact) while the PE is otherwise idle.
  - TensorE: 8 fp32 transposes back-to-back, then 4 bf16 matmuls
    (bf16 keeps fp32 PSUM accumulation; operand rounding gives ~2e-3
    relative error, well inside the 2e-2 gate, and enables fast
    weight load).
  - epilogue per tile: out_sb = psum + bias_bc (DVE), DMA out on
    alternating rings.
"""

import numpy as np

import concourse.bass as bass
import concourse.mybir as mybir
from concourse.bacc import Bacc
from concourse.bass import ts
from concourse.bass_utils import run_bass_kernel_spmd
from concourse.masks import make_identity
from concourse.tile import TileContext

N_CORES = 8
B, IN, OUT = 4096, 128, 512
B_SH = B // N_CORES  # 512 rows of x per core
P = 128
NT = B_SH // P  # 4 batch tiles per core
WT = OUT // P  # 4 weight tiles
F32 = mybir.dt.float32
F32R = mybir.dt.float32r
BF16 = mybir.dt.bfloat16

_CACHED_NC = None


def _build():
    nc = Bacc()

    x_ext = nc.declare_dram_parameter("x", [B_SH, IN], F32, isOutput=False)
    w_ext = nc.declare_dram_parameter("weight", [OUT, IN], F32, isOutput=False)
    b_ext = nc.declare_dram_parameter("bias", [OUT], F32, isOutput=False)
    out_ext = nc.declare_dram_parameter("out", [B_SH, OUT], F32, isOutput=True)

    # row r = 4p + a -> partition p, free block a (2KB contiguous / partition)
    x_blk = x_ext.rearrange("(p a) i -> p a i", a=NT)
    w_blk = w_ext.rearrange("(p a) i -> p a i", a=WT)
    out_blk = out_ext.rearrange("(p a) o -> p a o", a=NT)

    with TileContext(nc) as tc:
        with (
            tc.tile_pool(name="consts", bufs=1) as consts,
            tc.tile_pool(name="sbuf", bufs=1) as sbuf,
            tc.tile_pool(name="xtp", bufs=4) as xtp,
            tc.tile_pool(name="psum_b", bufs=1, space="PSUM") as psum_b_pool,
            tc.tile_pool(name="psum_w", bufs=1, space="PSUM") as psum_w_pool,
            tc.tile_pool(name="psum_x", bufs=2, space="PSUM") as psum_x_pool,
            tc.tile_pool(name="psum_out", bufs=3, space="PSUM") as psum_out,
            tc.tile_pool(name="outp", bufs=4) as outp,
        ):
            # ---- input DMAs ----
            w_nat = sbuf.tile([P, WT, P], F32)
            nc.sync.dma_start(w_nat, w_blk)
            bias_sb = consts.tile([1, OUT], F32)
            nc.scalar.dma_start(bias_sb, b_ext[None, :])
            x_nat = sbuf.tile([P, NT, P], F32)
            nc.scalar.dma_start(x_nat[:, 0:2, :], x_blk[:, 0:2, :])
            nc.scalar.dma_start(x_nat[:, 2:4, :], x_blk[:, 2:4, :])

            # ---- identity on GpSimd ----
            ident = consts.tile([P, P], F32)
            make_identity(nc, ident)

            # ---- bias broadcast via ones-outer-product matmul (exact) ----
            of = consts.tile([1, P], F32)
            nc.gpsimd.memset(of, 1.0)
            ones_r = consts.tile([1, P], F32R)
            nc.vector.tensor_copy(ones_r, of)
            bias_r = consts.tile([1, OUT], F32R)
            nc.vector.tensor_copy(bias_r, bias_sb)
            psum_b = psum_b_pool.tile([P, OUT], F32)
            nc.tensor.matmul(psum_b, ones_r, bias_r, start=True, stop=True)
            bias_bc = consts.tile([P, OUT], F32)
            nc.vector.tensor_copy(bias_bc, psum_b)

            # ---- 8 transposes back-to-back on PE ----
            psum_w = psum_w_pool.tile([P, WT * P], F32)
            for t in range(WT):
                nc.tensor.transpose(psum_w[:, ts(t, P)], w_nat[:, t, :], ident)
            pxts = []
            for t in range(NT):
                pxt = psum_x_pool.tile([P, P], F32)
                nc.tensor.transpose(pxt, x_nat[:, t, :], ident)
                pxts.append(pxt)

            # ---- casts to bf16 (DVE); wT read-AP undoes o = 4j + a ----
            wT = sbuf.tile([P, OUT], BF16)
            nc.vector.tensor_copy(
                wT.rearrange("p (j a) -> p j a", a=WT),
                psum_w.rearrange("p (a j) -> p j a", a=WT),
            )
            xTs = []
            for t in range(NT):
                xTt = xtp.tile([P, P], BF16)
                nc.vector.tensor_copy(xTt, pxts[t])
                xTs.append(xTt)

            # ---- main matmuls back-to-back + epilogue ----
            for t in range(NT):
                po = psum_out.tile([P, OUT], F32)
                nc.tensor.matmul(po, xTs[t], wT, start=True, stop=True)
                ot = outp.tile([P, OUT], F32)
                nc.vector.tensor_add(ot, po, bias_bc)
                eng = nc.sync if t % 2 == 0 else nc.scalar
                eng.dma_start(out_blk[:, t, :], ot)

    nc.finalize()
    return nc


def get_nc():
    global _CACHED_NC
    if _CACHED_NC is None:
        _CACHED_NC = _build()
    return _CACHED_NC


def make_in_maps(x, weight, bias, logits, gumbel):
    x = np.ascontiguousarray(x, dtype=np.float32)
    weight = np.ascontiguousarray(weight, dtype=np.float32)
    bias = np.ascontiguousarray(bias, dtype=np.float32)
    return [
        {
            "x": np.ascontiguousarray(x[i * B_SH : (i + 1) * B_SH]),
            "weight": weight,
            "bias": bias,
        }
        for i in range(N_CORES)
    ]


def run(inputs, trace=False, **kwargs):
    nc = get_nc()
    in_maps = make_in_maps(**inputs)
    res = run_bass_kernel_spmd(
        nc, in_maps, core_ids=list(range(N_CORES)), trace=trace, **kwargs
    )
    out = np.concatenate(
        [np.asarray(res.results[i]["out"]) for i in range(N_CORES)], axis=0
    )
    return out.astype(np.float32), res


def kernel(**inputs):
    out, _ = run(inputs, trace=False)
    return out


# revision 18
# speedup vs baseline: 1.1448x; 1.0609x over previous
"""Distributed Trainium2 kernel for nn_AtomicLinear.

Reference math:
    perm = softmax((logits + gumbel) / T, axis=-1)          # [128, 128]
    rowsum = perm.sum(-1)                                   # [128]
    out = einsum('bi,oi,i->bo', x, weight, rowsum) + bias   # [4096, 512]

softmax(z, axis=-1) rows sum to 1 by construction (the reference's own
rowsum is 1 +- 1e-7 float noise), so the contraction reduces exactly to
    out = x @ weight.T + bias.

Sharding: data-parallel over the batch axis of x -- each of the 8 cores
takes a 512-row shard of x, replicates weight/bias, and produces its
512-row shard of the output. No collectives.

Implementation notes:
  - x and weight are loaded with row group r = 4p + a on partition p /
    free block a: every per-partition DMA chunk is 2 KB contiguous.
    The row permutation is undone on the x side by a stride-4 row AP in
    the output DMA, and on the weight side by a free-dim-shuffled read
    AP in the wT cast.
  - sync ring: two weight halves (weight gates all matmuls), then the
    bias broadcast; scalar ring: two x halves.
  - TensorE: 8 fp32 transposes back-to-back, then 4 bf16 matmuls
    (fp32 PSUM accumulation; bf16 operand rounding gives ~2e-3 relative
    error, well inside the 2e-2 gate).
  - PSUM -> SBUF bf16 casts split between DVE and the otherwise-idle
    Scalar (ACT) engine.
  - epilogue per tile: out_sb = psum + bias_bc (DVE), DMA out on
    alternating rings.
"""

import numpy as np

import concourse.bass as bass
import concourse.mybir as mybir
from concourse.bacc import Bacc
from concourse.bass import ts
from concourse.bass_utils import run_bass_kernel_spmd
from concourse.masks import make_identity
from concourse.tile import TileContext

N_CORES = 8
B, IN, OUT = 4096, 128, 512
B_SH = B // N_CORES  # 512 rows of x per core
P = 128
NT = B_SH // P  # 4 batch tiles per core
WT = OUT // P  # 4 weight tiles
F32 = mybir.dt.float32
BF16 = mybir.dt.bfloat16

_CACHED_NC = None


def _build():
    nc = Bacc()

    x_ext = nc.declare_dram_parameter("x", [B_SH, IN], F32, isOutput=False)
    w_ext = nc.declare_dram_parameter("weight", [OUT, IN], F32, isOutput=False)
    b_ext = nc.declare_dram_parameter("bias", [OUT], F32, isOutput=False)
    out_ext = nc.declare_dram_parameter("out", [B_SH, OUT], F32, isOutput=True)

    # row r = 4p + a -> partition p, free block a (2KB contiguous / partition)
    x_blk = x_ext.rearrange("(p a) i -> p a i", a=NT)
    w_blk = w_ext.rearrange("(p a) i -> p a i", a=WT)
    out_blk = out_ext.rearrange("(p a) o -> p a o", a=NT)

    with TileContext(nc) as tc:
        with (
            tc.tile_pool(name="consts", bufs=1) as consts,
            tc.tile_pool(name="sbuf", bufs=1) as sbuf,
            tc.tile_pool(name="xtp", bufs=4) as xtp,
            tc.tile_pool(name="psum_w", bufs=1, space="PSUM") as psum_w_pool,
            tc.tile_pool(name="psum_x", bufs=4, space="PSUM") as psum_x_pool,
            tc.tile_pool(name="psum_out", bufs=3, space="PSUM") as psum_out,
            tc.tile_pool(name="outp", bufs=4) as outp,
        ):
            # ---- input DMAs ----
            w_nat = sbuf.tile([P, WT, P], F32)
            nc.sync.dma_start(w_nat[:, 0:2, :], w_blk[:, 0:2, :])
            nc.sync.dma_start(w_nat[:, 2:4, :], w_blk[:, 2:4, :])
            x_nat = sbuf.tile([P, NT, P], F32)
            nc.scalar.dma_start(x_nat[:, 0:2, :], x_blk[:, 0:2, :])
            nc.scalar.dma_start(x_nat[:, 2:4, :], x_blk[:, 2:4, :])
            bias_bc = consts.tile([P, OUT], F32)
            nc.sync.dma_start(bias_bc, b_ext[None, :].broadcast_to([P, OUT]))

            # ---- identity on GpSimd ----
            ident = consts.tile([P, P], F32)
            make_identity(nc, ident)

            # ---- 8 transposes back-to-back on PE ----
            psum_w = psum_w_pool.tile([P, WT * P], F32)
            for t in range(WT):
                nc.tensor.transpose(psum_w[:, ts(t, P)], w_nat[:, t, :], ident)
            pxts = []
            for t in range(NT):
                pxt = psum_x_pool.tile([P, P], F32)
                nc.tensor.transpose(pxt, x_nat[:, t, :], ident)
                pxts.append(pxt)

            # ---- casts to bf16; wT read-AP undoes o = 4j + a ----
            wT = sbuf.tile([P, OUT], BF16)
            nc.vector.tensor_copy(
                wT.rearrange("p (j a) -> p j a", a=WT),
                psum_w.rearrange("p (a j) -> p j a", a=WT),
            )
            xTs = []
            for t in range(NT):
                xTt = xtp.tile([P, P], BF16)
                if t % 2 == 0:
                    nc.vector.tensor_copy(xTt, pxts[t])
                else:
                    nc.scalar.copy(xTt, pxts[t])
                xTs.append(xTt)

            # ---- main matmuls back-to-back + epilogue ----
            for t in range(NT):
                po = psum_out.tile([P, OUT], F32)
                nc.tensor.matmul(po, xTs[t], wT, start=True, stop=True)
                ot = outp.tile([P, OUT], F32)
                nc.vector.tensor_add(ot, po, bias_bc)
                eng = nc.sync if t % 2 == 0 else nc.scalar
                eng.dma_start(out_blk[:, t, :], ot)

    nc.finalize()
    return nc


def get_nc():
    global _CACHED_NC
    if _CACHED_NC is None:
        _CACHED_NC = _build()
    return _CACHED_NC


def make_in_maps(x, weight, bias, logits, gumbel):
    x = np.ascontiguousarray(x, dtype=np.float32)
    weight = np.ascontiguousarray(weight, dtype=np.float32)
    bias = np.ascontiguousarray(bias, dtype=np.float32)
    return [
        {
            "x": np.ascontiguousarray(x[i * B_SH : (i + 1) * B_SH]),
            "weight": weight,
            "bias": bias,
        }
        for i in range(N_CORES)
    ]


def run(inputs, trace=False, **kwargs):
    nc = get_nc()
    in_maps = make_in_maps(**inputs)
    res = run_bass_kernel_spmd(
        nc, in_maps, core_ids=list(range(N_CORES)), trace=trace, **kwargs
    )
    out = np.concatenate(
        [np.asarray(res.results[i]["out"]) for i in range(N_CORES)], axis=0
    )
    return out.astype(np.float32), res


def kernel(**inputs):
    out, _ = run(inputs, trace=False)
    return out
